# revision 36
# baseline (speedup 1.0000x reference)
"""Trainium2 Bass kernel for the BottleNeck-involution block (pixel-major).

Sharding: pure data parallel over (batch=4) x (H halves) = 8 shards, one per
NeuronCore.  Each core computes a (1, 128, 48, 96) slice of the output.

Involution layout: PIXEL-MAJOR — the 96 output columns (j) sit on SBUF
partitions and (row i, channel c') on the free dim.  The per-pixel involution
kernel is then a per-partition tensor that broadcasts across the 16 channels
of each group via a stride-0 free-dim access pattern, so the 16x channel
broadcast (and all its PSUM->SBUF copies) disappears entirely.

Channels are host-permuted to c' = u*4 + g (group index innermost) so the
kern operand's broadcast dim is not the innermost AP dim, keeping the DVE
2-byte 2x mode.  Column shifts (dj) become partition shifts, which engines
cannot address (32-alignment rule), so the host x is loaded 7 times at
partition offsets 0..6; taps are emitted with Pool taps
spread evenly (and their accumulates deferred two slots) so the in-order PE
accumulate stream is never blocked by either producer.

Per-core pipeline:
  t       = relu(bn_r(w_reduce @ x))        PE + ACT   (channel-major)
  kern_pm = [t;1]^T @ [w_span; b_span]      PE         (pixel-major, per row)
  prod_kk = kern_pm(bcast) * x_shift        DVE / Pool (pixel-major, bf16 2x)
  acc     = b_i + sum_kk prod_kk            PE identity-matmul PSUM accum
                                            (3 row-segments: 24/16/8)
  x1      = gelu(acc)                       ACT        (bn_i folded: s_i into
                                                        x copies, b_i via init)
  x1_cm   = transpose(x1)                   PE transpose + ACT/DVE copies
  out     = gelu(wconv@x1 + wmap@x + btail) PE + ACT   (channel-major)
"""

import sys, os

sys.path.insert(0, "/opt/trn_rl_repo")
KPHASE = int(os.environ.get("KPHASE", "99"))
KDEBUG = int(os.environ.get("KDEBUG", "0"))

import numpy as np

import concourse.bass as bass
from concourse import bacc
import concourse.mybir as mybir
import concourse.tile as tile
from concourse.bass_utils import run_bass_kernel_spmd

F32 = mybir.dt.float32
BF16 = mybir.dt.bfloat16

EPS = 1e-5
KS = 7            # involution kernel size
KK = KS * KS      # 49 taps
GC = 16           # channels per involution group
G = 4             # groups
CR = 16           # reduced channels
B, C, H, W = 4, 64, 96, 96
CO = 128
NCORES = 8

ROWS = H // 2     # 48 output rows per core
PAD = 3
IROWS = ROWS + 2 * PAD   # 54 input rows (with halo)
JP = W + 2 * PAD         # 102 padded columns
HALF = 24
HFD = HALF * C

SEGS = [(0, 24), (24, 16), (40, 8)]   # (row0, nrows) acc segments

# taps run on Pool (gpsimd) instead of DVE (Pool tensor_tensor, 0.42 eff)
POOL_TAPS = [(ki, 1) for ki in range(7)] + [(0, 4), (3, 4), (6, 4), (1, 4)]
DVE_TAPS = [(ki, kj) for kj in (0, 2, 3, 5, 6) for ki in range(7)] +     [(ki, 4) for ki in (2, 4, 5)]
# interleave: Pool taps spread evenly through the emission order (keeps the
# in-order PE accumulate stream fed by both engines at their capacity ratio)
POOL_SLOTS = frozenset(
    round(2 + s * (KK - 3) / 10) for s in range(11)
)
TAP_ORDER = []
_d, _p = iter(DVE_TAPS), iter(POOL_TAPS)
for _slot in range(KK):
    TAP_ORDER.append(next(_p) if _slot in POOL_SLOTS else next(_d))

# blob f32-slot layout: name -> (width, partitions)
_REGIONS = [
    ("xpm", IROWS * C // 2, 102),   # x * s_i, pixel-major bf16 [102, 54*64]
    ("xcm", HALF * W // 2, 128),    # x channel-major bf16 [128, 24*96]
    ("xcmE", HALF * W // 2, 128),   # x even rows (at partitions 64-127)
    ("xcmO", HALF * W // 2, 64),    # x odd rows (at partitions 0-63)
    ("wce", CO // 2, 128),          # [wconv; wmap] stacked bf16 [128, 128]
    ("wco", CO // 2, 128),          # [wmap; wconv] stacked bf16 [128, 128]
    ("wspan", (KK * G) // 2, 17),   # [w_span; b_span] bf16 [17, 196]
    ("wredt", CR // 2, 128),        # w_reduce^T bf16 [128, 16] (both halves)
    ("screl", 2, 16),               # bn_r (scale, bias) f32 [16, 2]
    ("btail", 1, 128),              # tail gelu bias f32 [128, 1]
    ("birow", HFD // 2, 1),         # b_i row bf16 [1, 1536]
    ("ones", ROWS * W // 2, 1),     # ones bf16 [1, 4608] (t row + acc lhsT)
    ("iden", W // 2, 96),           # identity bf16 [96, 96]
]
_OFF = {}
_o = 0
for _name, _w, _p in _REGIONS:
    _OFF[_name] = (_o, _o + _w)
    _o += _w
BLOBW = _o

_CACHE = {}


def _build_bass():
    nc = bacc.Bacc()

    blob_d = nc.dram_tensor("blob", [128, BLOBW], F32, kind="ExternalInput")
    out_d = nc.dram_tensor("out", [CO, ROWS * W], F32, kind="ExternalOutput")
    if KDEBUG:
        tdbg_d = nc.dram_tensor("tdbg", [17, ROWS * W // 2], F32, kind="ExternalOutput")
        kdbg_d = nc.dram_tensor("kdbg", [96, ROWS * KK * G // 2], F32, kind="ExternalOutput")
        xdbg_d = nc.dram_tensor("xdbg", [96, ROWS * C // 2], F32, kind="ExternalOutput")

    def breg(name, p0=0, p1=None):
        a, b_ = _OFF[name]
        np_ = dict((n, pp) for n, _, pp in _REGIONS)[name] if p1 is None else p1
        return blob_d[p0:np_, a:b_]

    with tile.TileContext(nc) as tc:
        with (
            tc.tile_pool(name="work", bufs=1) as work,
            tc.tile_pool(name="prodD", bufs=12) as prodD,
            tc.tile_pool(name="prodP", bufs=8) as prodP,
            tc.tile_pool(name="outp", bufs=4) as outp,
        ):
            # ---- SBUF tiles + loads (ordered by first use) ----
            wredt = work.tile([128, CR], BF16, name="wredt")
            nc.sync.dma_start(wredt[:].bitcast(F32), breg("wredt"))
            screl = work.tile([16, 2], F32, name="screl")
            nc.sync.dma_start(screl[:], breg("screl"))
            x_cm = work.tile([128, HALF * W], BF16, name="x_cm")
            nc.sync.dma_start(x_cm[0:64, :].bitcast(F32), breg("xcm", 0, 64))
            nc.sync.dma_start(x_cm[64:128, :].bitcast(F32), breg("xcm", 64, 128))
            wspan = work.tile([17, KK * G], BF16, name="wspan")
            nc.sync.dma_start(wspan[:].bitcast(F32), breg("wspan"))
            t_sb = work.tile([17, ROWS * W], BF16, name="t_sb")
            nc.sync.dma_start(t_sb[16:17, :].bitcast(F32), breg("ones"))
            birow = work.tile([1, HFD], BF16, name="birow")
            nc.sync.dma_start(birow[:].bitcast(F32), breg("birow"))
            ones1 = work.tile([1, W], BF16, name="ones1")
            nc.sync.dma_start(
                ones1[:].bitcast(F32), breg("ones", 0, 1)[0:1, 0:W // 2]
            )
            iden = work.tile([96, W], BF16, name="iden")
            nc.sync.dma_start(iden[:].bitcast(F32), breg("iden"))
            btail = work.tile([128, 1], F32, name="btail")
            nc.sync.dma_start(btail[:], breg("btail"))

            # 7 partition-shifted copies of pixel-major x (one per kj),
            # in kj order so tap groups can start as each copy lands
            xs = []
            for kj in range(KS):
                t_ = work.tile([96, IROWS * C], BF16, name=f"xs{kj}")
                nc.sync.dma_start(
                    t_[:].bitcast(F32),
                    blob_d[kj:kj + 96, _OFF["xpm"][0]:_OFF["xpm"][1]],
                )
                xs.append(t_)

            wce = work.tile([128, CO], BF16, name="wce")
            nc.sync.dma_start(wce[:].bitcast(F32), breg("wce"))
            wco = work.tile([128, CO], BF16, name="wco")
            nc.sync.dma_start(wco[:].bitcast(F32), breg("wco"))
            # mixed static/dynamic tail rhs tiles:
            #   rhsE = [x1 even rows (ACT) | x even rows (DMA)]
            #   rhsO = [x odd rows (DMA)   | x1 odd rows (ACT)]
            rhsE = work.tile([128, HALF * W], BF16, name="rhsE")
            a0, a1 = _OFF["xcmE"]
            nc.sync.dma_start(
                rhsE[64:128, :].bitcast(F32), blob_d[64:128, a0:a1]
            )
            rhsO = work.tile([128, HALF * W], BF16, name="rhsO")
            nc.sync.dma_start(rhsO[0:64, :].bitcast(F32), breg("xcmO"))

            kern_pm = work.tile([96, ROWS * KK * G], BF16, name="kern_pm")
            x1g = work.tile([96, ROWS * C], BF16, name="x1g")

            # ---- PSUM pools (LIFO): kpsB > kpsA > tpsp, then acc > tail ----
            kpsB_cm = tc.tile_pool(name="kpsB", bufs=1, space="PSUM")
            kpsB = kpsB_cm.__enter__()
            kpsA_cm = tc.tile_pool(name="kpsA", bufs=3, space="PSUM")
            kpsA = kpsA_cm.__enter__()
            tps_cm = tc.tile_pool(name="tpsp", bufs=2, space="PSUM")
            tpsp = tps_cm.__enter__()

            def t_chunk(blk, ci):
                """t rows [24*blk + 4*ci, +4)."""
                tp = tpsp.tile([16, 512], F32, name="tps", tag="tps")
                nc.tensor.matmul(
                    tp[:, 0:4 * W],
                    wredt[64 * blk:64 * blk + 64, :],
                    x_cm[64 * blk:64 * blk + 64, ci * 4 * W:(ci + 1) * 4 * W],
                    start=True, stop=True,
                )
                nc.scalar.activation(
                    t_sb[0:16, (24 * blk + 4 * ci) * W:(24 * blk + 4 * ci + 4) * W],
                    tp[:, 0:4 * W],
                    mybir.ActivationFunctionType.Relu,
                    bias=screl[:, 1:2],
                    scale=screl[:, 0:1],
                )

            def kern_pair(ip, pool, dve_copy):
                """produce kern_pm rows 2ip, 2ip+1 (196 cols each)."""
                kp = pool.tile([96, 512], F32, name="kps", tag="kps")
                for s in range(2):
                    i = 2 * ip + s
                    nc.tensor.matmul(
                        kp[:, s * KK * G:(s + 1) * KK * G],
                        t_sb[0:17, i * W:(i + 1) * W],
                        wspan[:],
                        start=True, stop=True,
                    )
                dst = kern_pm[:, 2 * ip * KK * G:(2 * ip + 2) * KK * G]
                if dve_copy:
                    nc.vector.tensor_copy(dst, kp[:, 0:2 * KK * G])
                else:
                    nc.scalar.activation(
                        dst, kp[:, 0:2 * KK * G],
                        mybir.ActivationFunctionType.Identity,
                    )

            # lead-in: t rows 0-24 with kern rows 0-23 chasing (copies
            # split DVE/ACT), then remaining t
            t_chunk(0, 0)
            t_chunk(0, 1)
            for ip in range(4):
                kern_pair(ip, kpsA, dve_copy=(ip % 2 == 0))
                if ip % 2 == 1:
                    t_chunk(0, 2 + ip // 2)
            for ip in range(4, 8):
                kern_pair(ip, kpsA, dve_copy=(ip % 2 == 0))
                if ip % 2 == 1:
                    t_chunk(0, 4 + (ip - 4) // 2)
            t_chunk(1, 0)
            for ip in range(8, 12):
                kern_pair(ip, kpsA, dve_copy=(ip % 2 == 0))
                if ip % 2 == 1:
                    t_chunk(1, 1 + (ip - 8) // 2)
            t_chunk(1, 3)
            t_chunk(1, 4)
            t_chunk(1, 5)
            tps_cm.__exit__(None, None, None)
            kpsA_cm.__exit__(None, None, None)

            acc_cm = tc.tile_pool(name="accp", bufs=1, space="PSUM")
            accp = acc_cm.__enter__()
            tail_cm = tc.tile_pool(name="tailp", bufs=1, space="PSUM")
            tailp = tail_cm.__enter__()
            tp2_cm = tc.tile_pool(name="tp2p", bufs=1, space="PSUM")
            tp2p = tp2_cm.__enter__()
            ost_holder = [None]
            x1c_q = {}

            def tail_a(ci, x1c_dve):
                """transpose + copy for global row pair (2ci, 2ci+1)."""
                tp2 = tp2p.tile([128, 1024], BF16, name="tp2", tag="tp2")
                nc.tensor.transpose(
                    tp2[:, 0:W], x1g[:, ci * 2 * C:(ci + 1) * 2 * C], iden[:]
                )
                re = rhsE[0:64, ci * W:(ci + 1) * W]
                ro = rhsO[64:128, ci * W:(ci + 1) * W]
                if x1c_dve:
                    nc.vector.tensor_copy(re, tp2[0:64, 0:W])
                    nc.vector.tensor_copy(ro, tp2[64:128, 0:W])
                else:
                    nc.scalar.activation(
                        re, tp2[0:64, 0:W],
                        mybir.ActivationFunctionType.Identity,
                    )
                    nc.scalar.activation(
                        ro, tp2[64:128, 0:W],
                        mybir.ActivationFunctionType.Identity,
                    )
                x1c_q[ci] = True

            def tail_b(ci):
                """1x1 convs + gelu + out DMA for global row pair."""
                x1c_q.pop(ci)
                if KPHASE < 72:
                    return
                tl = tailp.tile([128, 512], F32, name="tailps", tag="tailps")
                nc.tensor.matmul(
                    tl[:, 0:W], wce[:], rhsE[:, ci * W:(ci + 1) * W],
                    start=True, stop=True,
                )
                nc.tensor.matmul(
                    tl[:, W:2 * W], wco[:], rhsO[:, ci * W:(ci + 1) * W],
                    start=True, stop=True,
                )
                if KPHASE < 73:
                    return
                if ci % 2 == 0:
                    ost_holder[0] = outp.tile(
                        [128, 4 * W], F32, name="ost", tag="ost"
                    )
                ost = ost_holder[0]
                nc.scalar.activation(
                    ost[:, (ci % 2) * 2 * W:((ci % 2) + 1) * 2 * W],
                    tl[:, 0:2 * W],
                    mybir.ActivationFunctionType.Gelu,
                    bias=btail[:],
                    scale=1.0,
                )
                if ci % 2 == 1 and KPHASE >= 74:
                    r0 = 2 * (ci - 1)
                    nc.sync.dma_start(out_d[:, r0 * W:(r0 + 4) * W], ost[:])

            # ---- segments: products + accumulate + gelu ----
            seg_list = SEGS if KPHASE >= 3 else []
            # 49 taps per segment in kj-major order; interleaved extras:
            #   seg0 -> kern rows 24-47; seg1 -> tails of rows 0-23;
            #   seg2 -> tails of rows 24-39; trailing -> tails of rows 40-47
            for s, (r0, nr) in enumerate(seg_list):
                fd = nr * C
                atag = "accA" if s % 2 == 0 else "accB"
                acc = accp.tile([96, fd], F32, name=atag, tag=atag)
                for c0 in range(0, fd, 512):
                    nc.tensor.matmul(
                        acc[:, c0:c0 + 512], ones1[:],
                        birow[:, c0:c0 + 512], start=True, stop=False,
                    )
                acc_pending = []
                lag = [0]

                def flush_acc(limit):
                    while acc_pending and len(acc_pending) > limit:
                        pr_, last_ = acc_pending.pop(0)
                        for c0 in range(0, fd, 512):
                            nc.tensor.matmul(
                                acc[:, c0:c0 + 512], iden[:],
                                pr_[:, c0:c0 + 512],
                                start=False, stop=last_,
                            )

                for tapn in range(KK):
                    ki, kj = TAP_ORDER[tapn]
                    kk = ki * KS + kj
                    pool_tap = tapn in POOL_SLOTS and KPHASE >= 4
                    pp = prodP if pool_tap else prodD
                    pr = pp.tile([96, fd], BF16, name="prod", tag="prod")
                    pr4 = pr[:].rearrange("p (i u g) -> p i u g", i=nr, u=GC)
                    in0 = xs[kj][:].rearrange(
                        "p (i c) -> p i c", i=IROWS
                    )[:, r0 + ki:r0 + ki + nr, :].rearrange(
                        "p i (u g) -> p i u g", g=G
                    )
                    in1 = kern_pm[:].rearrange(
                        "p (i k) -> p i k", k=KK * G
                    )[:, r0:r0 + nr, kk * G:(kk + 1) * G]
                    in1b = in1.unsqueeze(2).to_broadcast([96, nr, GC, G])
                    if pool_tap:
                        nc.gpsimd.tensor_tensor(
                            out=pr4, in0=in0, in1=in1b,
                            op=mybir.AluOpType.mult,
                        )
                    else:
                        nc.vector.tensor_tensor(
                            out=pr4, in0=in0, in1=in1b,
                            op=mybir.AluOpType.mult,
                        )
                    # defer pool-tap accs ~2 slots so the in-order PE
                    # stream is not blocked by Pool's longer op latency
                    acc_pending.append((pr, tapn == KK - 1))
                    if pool_tap and int(os.environ.get("KLAG", "1")):
                        lag[0] = 2
                    flush_acc(lag[0])
                    if lag[0] > 0 and not pool_tap:
                        lag[0] -= 1
                    # interleaved kern production rows 24-47 (seg 0)
                    if (s == 0 and tapn % 4 == 1 and tapn // 4 < 12
                            and KPHASE >= 5):
                        kern_pair(12 + tapn // 4, kpsB, dve_copy=False)
                    # interleaved, phase-split tail chunks (segs 1-2)
                    if s == 1 and KPHASE >= 71:
                        if tapn % 4 == 1 and tapn // 4 < 12:
                            tail_a(tapn // 4, x1c_dve=False)
                        if tapn % 4 == 3 and tapn // 4 < 12:
                            tail_b(tapn // 4)
                    if s == 2 and KPHASE >= 71:
                        if tapn % 6 == 1 and tapn // 6 < 8:
                            tail_a(12 + tapn // 6, x1c_dve=False)
                        if tapn % 6 == 4 and tapn // 6 < 8:
                            tail_b(12 + tapn // 6)
                flush_acc(0)
                if KPHASE >= 6:
                    nc.scalar.activation(
                        x1g[:, r0 * C:(r0 + nr) * C],
                        acc[:],
                        mybir.ActivationFunctionType.Gelu,
                    )
            # trailing tail chunks (rows 40-47)
            if KPHASE >= 71:
                for ci in range(20, 24):
                    tail_a(ci, x1c_dve=True)
                    tail_b(ci)
            if KDEBUG:
                nc.sync.dma_start(tdbg_d[:, :], t_sb[:].bitcast(F32))
                nc.sync.dma_start(kdbg_d[:, :], kern_pm[:].bitcast(F32))
                nc.sync.dma_start(xdbg_d[:, :], x1g[:].bitcast(F32))
            tp2_cm.__exit__(None, None, None)
            tail_cm.__exit__(None, None, None)
            acc_cm.__exit__(None, None, None)
            kpsB_cm.__exit__(None, None, None)

    if not nc.is_finalized():
        nc.finalize()
    return nc


def _bf16_pack(arr):
    """bf16-cast a [P, N] array and pack into [P, N/2] f32 slots."""
    import ml_dtypes

    a = np.ascontiguousarray(np.asarray(arr, np.float32)).astype(ml_dtypes.bfloat16)
    return a.view(np.float32)


def _prep_blob_consts(w_reduce, g_r, b_r, m_r, v_r, w_span, b_span,
                      g_i, b_i, m_i, v_i, w_conv, g_c, b_c, m_c, v_c,
                      w_map, b_map, g_m, b_m, m_m, v_m, perm):
    f = np.float32

    def bn_fold(g, b, m, v):
        s = g / np.sqrt(v + EPS)
        return s.astype(f), (b - m * s).astype(f)

    sc_r, bi_r = bn_fold(g_r, b_r, m_r, v_r)
    sc_i, bi_i = bn_fold(g_i, b_i, m_i, v_i)
    sc_c, bi_c = bn_fold(g_c, b_c, m_c, v_c)
    sc_m, bi_m = bn_fold(g_m, b_m, m_m, v_m)

    cb = np.zeros((128, BLOBW), f)

    def put(name, arr, packed=False):
        a, b_ = _OFF[name]
        arr = np.asarray(arr, f)
        v = _bf16_pack(arr) if packed else arr
        cb[0:v.shape[0], a:a + v.shape[1]] = v

    # wspan_aug [17, 196]: rows 0-15 w_span, row 16 b_span; col = kk*4+g
    wsa = np.zeros((17, KK * G), f)
    ws3 = w_span.reshape(G, KK, CR)          # [g, kk, r]
    wsa[0:16] = ws3.transpose(2, 1, 0).reshape(CR, KK * G)
    wsa[16] = b_span.reshape(G, KK).T.reshape(KK * G)
    put("wspan", wsa, packed=True)

    # w_reduce^T with c' permutation, same 16 cols for both 64-partition halves
    wrt = np.zeros((128, CR), f)
    wrt[0:64] = w_reduce[:, perm].T
    wrt[64:128] = w_reduce[:, perm].T
    put("wredt", wrt, packed=True)
    put("screl", np.stack([sc_r, bi_r], axis=1))

    wc = (w_conv[:, perm] * sc_c[:, None]).T
    wm = (w_map[:, perm] * sc_m[:, None]).T
    put("wce", np.concatenate([wc, wm], axis=0), packed=True)
    put("wco", np.concatenate([wm, wc], axis=0), packed=True)
    put("btail", (bi_c + sc_m * b_map + bi_m)[:, None])
    put("birow", np.tile(bi_i[perm], HALF)[None, :], packed=True)
    put("ones", np.ones((1, ROWS * W), f), packed=True)
    put("iden", np.eye(96, dtype=f), packed=True)
    return cb, sc_i


def kernel(**inputs):
    x = np.asarray(inputs["x"], dtype=np.float32)
    assert x.shape == (B, C, H, W)

    # channel permutation c' = u*4 + g  (group innermost)
    perm = np.array([(cp // G) + GC * (cp % G) for cp in range(C)], np.int64)

    if "cb" not in _CACHE:
        cb, sc_i = _prep_blob_consts(
            **{k: np.asarray(v) for k, v in inputs.items() if k != "x"}, perm=perm
        )
        _CACHE["cb"] = cb
        _CACHE["sc_i"] = sc_i
    cb, sc_i = _CACHE["cb"], _CACHE["sc_i"]

    if "nc" not in _CACHE:
        _CACHE["nc"] = _build_bass()
    nc = _CACHE["nc"]

    xp = x[:, perm, :, :]                       # [B, c', H, W]
    xs_scaled = xp * sc_i[perm][None, :, None, None]

    in_maps = []
    for core in range(NCORES):
        b, half = core // 2, core % 2
        r0 = half * ROWS
        blob = cb.copy()
        # pixel-major x*s_i: [102 j, (54 i, 64 c')]
        xpm = np.zeros((JP, IROWS, C), np.float32)
        glo, ghi = max(r0 - PAD, 0), min(r0 + ROWS + PAD, H)
        xpm[PAD:PAD + W, glo - (r0 - PAD):ghi - (r0 - PAD), :] = (
            xs_scaled[b, :, glo:ghi, :].transpose(2, 1, 0)
        )
        a0, a1 = _OFF["xpm"]
        blob[0:JP, a0:a1] = _bf16_pack(xpm.reshape(JP, IROWS * C))
        # channel-major x: [128 = (2 half-blocks, 64 c'), 24*96]
        xcm = np.empty((128, HALF * W), np.float32)
        xcm[0:64] = xp[b, :, r0:r0 + HALF, :].reshape(C, HALF * W)
        xcm[64:128] = xp[b, :, r0 + HALF:r0 + ROWS, :].reshape(C, HALF * W)
        a0, a1 = _OFF["xcm"]
        blob[0:128, a0:a1] = _bf16_pack(xcm)
        # row-parity x for the tail map branch (E at partitions 64-127)
        xr = xp[b, :, r0:r0 + ROWS, :]              # [c', 48, 96]
        a0, a1 = _OFF["xcmE"]
        blob[64:128, a0:a1] = _bf16_pack(xr[:, 0::2, :].reshape(C, HALF * W))
        a0, a1 = _OFF["xcmO"]
        blob[0:64, a0:a1] = _bf16_pack(xr[:, 1::2, :].reshape(C, HALF * W))
        in_maps.append({"blob": blob})

    res = run_bass_kernel_spmd(nc, in_maps, core_ids=list(range(NCORES)))

    out = np.empty((B, CO, H, W), np.float32)
    for core in range(NCORES):
        b, half = core // 2, core % 2
        o = np.asarray(res.results[core]["out"]).astype(np.float32)
        out[b, :, half * ROWS:(half + 1) * ROWS, :] = o.reshape(CO, ROWS, W)
    return out


# revision 39
# speedup vs baseline: 1.0010x; 1.0010x over previous
"""Trainium2 Bass kernel for the BottleNeck-involution block (pixel-major).

Sharding: pure data parallel over (batch=4) x (H halves) = 8 shards, one per
NeuronCore.  Each core computes a (1, 128, 48, 96) slice of the output.

Involution layout: PIXEL-MAJOR — the 96 output columns (j) sit on SBUF
partitions and (row i, channel c') on the free dim.  The per-pixel involution
kernel is then a per-partition tensor that broadcasts across the 16 channels
of each group via a stride-0 free-dim access pattern, so the 16x channel
broadcast (and all its PSUM->SBUF copies) disappears entirely.

Channels are host-permuted to c' = u*4 + g (group index innermost) so the
kern operand's broadcast dim is not the innermost AP dim, keeping the DVE
2-byte 2x mode.  Column shifts (dj) become partition shifts, which engines
cannot address (32-alignment rule), so the host x is loaded 7 times at
partition offsets 0..6; taps are emitted with Pool taps
spread evenly (and their accumulates deferred two slots) so the in-order PE
accumulate stream is never blocked by either producer.

Per-core pipeline:
  t       = relu(bn_r(w_reduce @ x))        PE + ACT   (channel-major)
  kern_pm = [t;1]^T @ [w_span; b_span]      PE         (pixel-major, per row)
  prod_kk = kern_pm(bcast) * x_shift        DVE / Pool (pixel-major, bf16 2x)
  acc     = b_i + sum_kk prod_kk            PE identity-matmul PSUM accum
                                            (3 row-segments: 24/16/8)
  x1      = gelu(acc)                       ACT        (bn_i folded: s_i into
                                                        x copies, b_i via init)
  x1_cm   = transpose(x1)                   PE transpose + ACT/DVE copies
  out     = gelu(wconv@x1 + wmap@x + btail) PE + ACT   (channel-major)
"""

import sys, os

sys.path.insert(0, "/opt/trn_rl_repo")
KPHASE = int(os.environ.get("KPHASE", "99"))
KDEBUG = int(os.environ.get("KDEBUG", "0"))

import numpy as np

import concourse.bass as bass
from concourse import bacc
import concourse.mybir as mybir
import concourse.tile as tile
from concourse.bass_utils import run_bass_kernel_spmd

F32 = mybir.dt.float32
BF16 = mybir.dt.bfloat16

EPS = 1e-5
KS = 7            # involution kernel size
KK = KS * KS      # 49 taps
GC = 16           # channels per involution group
G = 4             # groups
CR = 16           # reduced channels
B, C, H, W = 4, 64, 96, 96
CO = 128
NCORES = 8

ROWS = H // 2     # 48 output rows per core
PAD = 3
IROWS = ROWS + 2 * PAD   # 54 input rows (with halo)
JP = W + 2 * PAD         # 102 padded columns
HALF = 24
HFD = HALF * C

SEGS = [(0, 24), (24, 16), (40, 8)]   # (row0, nrows) acc segments

# taps run on Pool (gpsimd) instead of DVE (Pool tensor_tensor, 0.42 eff)
POOL_TAPS = [(ki, 1) for ki in range(7)] + [(0, 4), (3, 4), (6, 4), (1, 4)]
DVE_TAPS = [(ki, kj) for kj in (0, 2, 3, 5, 6) for ki in range(7)] +     [(ki, 4) for ki in (2, 4, 5)]
# interleave: Pool taps spread evenly through the emission order (keeps the
# in-order PE accumulate stream fed by both engines at their capacity ratio)
POOL_SLOTS = frozenset(
    round(2 + s * (KK - 3) / 10) for s in range(11)
)
TAP_ORDER = []
_d, _p = iter(DVE_TAPS), iter(POOL_TAPS)
for _slot in range(KK):
    TAP_ORDER.append(next(_p) if _slot in POOL_SLOTS else next(_d))

# blob f32-slot layout: name -> (width, partitions)
_REGIONS = [
    ("xpm", IROWS * C // 2, 102),   # x * s_i, pixel-major bf16 [102, 54*64]
    ("xcm", HALF * W // 2, 128),    # x channel-major bf16 [128, 24*96]
    ("xcmE", HALF * W // 2, 128),   # x even rows (at partitions 64-127)
    ("xcmO", HALF * W // 2, 64),    # x odd rows (at partitions 0-63)
    ("wce", CO // 2, 128),          # [wconv; wmap] stacked bf16 [128, 128]
    ("wco", CO // 2, 128),          # [wmap; wconv] stacked bf16 [128, 128]
    ("wspan", (KK * G) // 2, 17),   # [w_span; b_span] bf16 [17, 196]
    ("wredt", CR // 2, 128),        # w_reduce^T bf16 [128, 16] (both halves)
    ("screl", 2, 16),               # bn_r (scale, bias) f32 [16, 2]
    ("btail", 1, 128),              # tail gelu bias f32 [128, 1]
    ("birow", HFD // 2, 1),         # b_i row bf16 [1, 1536]
    ("ones", ROWS * W // 2, 1),     # ones bf16 [1, 4608] (t row + acc lhsT)
    ("iden", W // 2, 96),           # identity bf16 [96, 96]
]
_OFF = {}
_o = 0
for _name, _w, _p in _REGIONS:
    _OFF[_name] = (_o, _o + _w)
    _o += _w
BLOBW = _o

_CACHE = {}


def _build_bass():
    nc = bacc.Bacc()

    blob_d = nc.dram_tensor("blob", [128, BLOBW], F32, kind="ExternalInput")
    out_d = nc.dram_tensor("out", [CO, ROWS * W], F32, kind="ExternalOutput")
    if KDEBUG:
        tdbg_d = nc.dram_tensor("tdbg", [17, ROWS * W // 2], F32, kind="ExternalOutput")
        kdbg_d = nc.dram_tensor("kdbg", [96, ROWS * KK * G // 2], F32, kind="ExternalOutput")
        xdbg_d = nc.dram_tensor("xdbg", [96, ROWS * C // 2], F32, kind="ExternalOutput")

    def breg(name, p0=0, p1=None):
        a, b_ = _OFF[name]
        np_ = dict((n, pp) for n, _, pp in _REGIONS)[name] if p1 is None else p1
        return blob_d[p0:np_, a:b_]

    with tile.TileContext(nc) as tc:
        with (
            tc.tile_pool(name="work", bufs=1) as work,
            tc.tile_pool(name="prodD", bufs=12) as prodD,
            tc.tile_pool(name="prodP", bufs=8) as prodP,
            tc.tile_pool(name="outp", bufs=4) as outp,
        ):
            # ---- SBUF tiles + loads (ordered by first use) ----
            wredt = work.tile([128, CR], BF16, name="wredt")
            nc.sync.dma_start(wredt[:].bitcast(F32), breg("wredt"))
            screl = work.tile([16, 2], F32, name="screl")
            nc.sync.dma_start(screl[:], breg("screl"))
            x_cm = work.tile([128, HALF * W], BF16, name="x_cm")
            nc.sync.dma_start(x_cm[0:64, :].bitcast(F32), breg("xcm", 0, 64))
            nc.sync.dma_start(x_cm[64:128, :].bitcast(F32), breg("xcm", 64, 128))
            wspan = work.tile([17, KK * G], BF16, name="wspan")
            nc.sync.dma_start(wspan[:].bitcast(F32), breg("wspan"))
            t_sb = work.tile([17, ROWS * W], BF16, name="t_sb")
            nc.sync.dma_start(t_sb[16:17, :].bitcast(F32), breg("ones"))
            birow = work.tile([1, HFD], BF16, name="birow")
            nc.sync.dma_start(birow[:].bitcast(F32), breg("birow"))
            ones1 = work.tile([1, W], BF16, name="ones1")
            nc.sync.dma_start(
                ones1[:].bitcast(F32), breg("ones", 0, 1)[0:1, 0:W // 2]
            )
            iden = work.tile([96, W], BF16, name="iden")
            nc.sync.dma_start(iden[:].bitcast(F32), breg("iden"))
            btail = work.tile([128, 1], F32, name="btail")
            nc.sync.dma_start(btail[:], breg("btail"))

            # 7 partition-shifted copies of pixel-major x (one per kj),
            # in kj order so tap groups can start as each copy lands
            xs = []
            for kj in range(KS):
                t_ = work.tile([96, IROWS * C], BF16, name=f"xs{kj}")
                nc.sync.dma_start(
                    t_[:].bitcast(F32),
                    blob_d[kj:kj + 96, _OFF["xpm"][0]:_OFF["xpm"][1]],
                )
                xs.append(t_)

            wce = work.tile([128, CO], BF16, name="wce")
            nc.sync.dma_start(wce[:].bitcast(F32), breg("wce"))
            wco = work.tile([128, CO], BF16, name="wco")
            nc.sync.dma_start(wco[:].bitcast(F32), breg("wco"))
            # mixed static/dynamic tail rhs tiles:
            #   rhsE = [x1 even rows (ACT) | x even rows (DMA)]
            #   rhsO = [x odd rows (DMA)   | x1 odd rows (ACT)]
            rhsE = work.tile([128, HALF * W], BF16, name="rhsE")
            a0, a1 = _OFF["xcmE"]
            nc.sync.dma_start(
                rhsE[64:128, :].bitcast(F32), blob_d[64:128, a0:a1]
            )
            rhsO = work.tile([128, HALF * W], BF16, name="rhsO")
            nc.sync.dma_start(rhsO[0:64, :].bitcast(F32), breg("xcmO"))

            kern_pm = work.tile([96, ROWS * KK * G], BF16, name="kern_pm")
            x1g = work.tile([96, ROWS * C], BF16, name="x1g")

            # ---- PSUM pools (LIFO): kpsB > kpsA > tpsp, then acc > tail ----
            kpsB_cm = tc.tile_pool(name="kpsB", bufs=1, space="PSUM")
            kpsB = kpsB_cm.__enter__()
            kpsA_cm = tc.tile_pool(name="kpsA", bufs=3, space="PSUM")
            kpsA = kpsA_cm.__enter__()
            tps_cm = tc.tile_pool(name="tpsp", bufs=3, space="PSUM")
            tpsp = tps_cm.__enter__()

            def t_chunk(blk, ci):
                """t rows [24*blk + 4*ci, +4)."""
                tp = tpsp.tile([16, 512], F32, name="tps", tag="tps")
                nc.tensor.matmul(
                    tp[:, 0:4 * W],
                    wredt[64 * blk:64 * blk + 64, :],
                    x_cm[64 * blk:64 * blk + 64, ci * 4 * W:(ci + 1) * 4 * W],
                    start=True, stop=True,
                )
                nc.scalar.activation(
                    t_sb[0:16, (24 * blk + 4 * ci) * W:(24 * blk + 4 * ci + 4) * W],
                    tp[:, 0:4 * W],
                    mybir.ActivationFunctionType.Relu,
                    bias=screl[:, 1:2],
                    scale=screl[:, 0:1],
                )

            def kern_pair(ip, pool, dve_copy):
                """produce kern_pm rows 2ip, 2ip+1 (196 cols each)."""
                kp = pool.tile([96, 512], F32, name="kps", tag="kps")
                for s in range(2):
                    i = 2 * ip + s
                    nc.tensor.matmul(
                        kp[:, s * KK * G:(s + 1) * KK * G],
                        t_sb[0:17, i * W:(i + 1) * W],
                        wspan[:],
                        start=True, stop=True,
                    )
                dst = kern_pm[:, 2 * ip * KK * G:(2 * ip + 2) * KK * G]
                if dve_copy:
                    nc.vector.tensor_copy(dst, kp[:, 0:2 * KK * G])
                else:
                    nc.scalar.activation(
                        dst, kp[:, 0:2 * KK * G],
                        mybir.ActivationFunctionType.Identity,
                    )

            # lead-in: t rows 0-24 with kern rows 0-23 chasing (copies
            # split DVE/ACT), then remaining t
            t_chunk(0, 0)
            t_chunk(0, 1)
            for ip in range(4):
                kern_pair(ip, kpsA, dve_copy=(ip % 2 == 0))
                if ip % 2 == 1:
                    t_chunk(0, 2 + ip // 2)
            for ip in range(4, 8):
                kern_pair(ip, kpsA, dve_copy=(ip % 2 == 0))
                if ip % 2 == 1:
                    t_chunk(0, 4 + (ip - 4) // 2)
            t_chunk(1, 0)
            for ip in range(8, 12):
                kern_pair(ip, kpsA, dve_copy=(ip % 2 == 0))
                if ip % 2 == 1:
                    t_chunk(1, 1 + (ip - 8) // 2)
            t_chunk(1, 3)
            t_chunk(1, 4)
            t_chunk(1, 5)
            tps_cm.__exit__(None, None, None)
            kpsA_cm.__exit__(None, None, None)

            acc_cm = tc.tile_pool(name="accp", bufs=1, space="PSUM")
            accp = acc_cm.__enter__()
            tail_cm = tc.tile_pool(name="tailp", bufs=1, space="PSUM")
            tailp = tail_cm.__enter__()
            tp2_cm = tc.tile_pool(name="tp2p", bufs=1, space="PSUM")
            tp2p = tp2_cm.__enter__()
            ost_holder = [None]
            x1c_q = {}

            def tail_a(ci, x1c_dve):
                """transpose + copy for global row pair (2ci, 2ci+1)."""
                tp2 = tp2p.tile([128, 1024], BF16, name="tp2", tag="tp2")
                nc.tensor.transpose(
                    tp2[:, 0:W], x1g[:, ci * 2 * C:(ci + 1) * 2 * C], iden[:]
                )
                re = rhsE[0:64, ci * W:(ci + 1) * W]
                ro = rhsO[64:128, ci * W:(ci + 1) * W]
                if x1c_dve:
                    nc.vector.tensor_copy(re, tp2[0:64, 0:W])
                    nc.vector.tensor_copy(ro, tp2[64:128, 0:W])
                else:
                    nc.scalar.activation(
                        re, tp2[0:64, 0:W],
                        mybir.ActivationFunctionType.Identity,
                    )
                    nc.scalar.activation(
                        ro, tp2[64:128, 0:W],
                        mybir.ActivationFunctionType.Identity,
                    )
                x1c_q[ci] = True

            def tail_b(ci):
                """1x1 convs + gelu + out DMA for global row pair."""
                x1c_q.pop(ci)
                if KPHASE < 72:
                    return
                tl = tailp.tile([128, 512], F32, name="tailps", tag="tailps")
                nc.tensor.matmul(
                    tl[:, 0:W], wce[:], rhsE[:, ci * W:(ci + 1) * W],
                    start=True, stop=True,
                )
                nc.tensor.matmul(
                    tl[:, W:2 * W], wco[:], rhsO[:, ci * W:(ci + 1) * W],
                    start=True, stop=True,
                )
                if KPHASE < 73:
                    return
                if ci % 2 == 0:
                    ost_holder[0] = outp.tile(
                        [128, 4 * W], F32, name="ost", tag="ost"
                    )
                ost = ost_holder[0]
                nc.scalar.activation(
                    ost[:, (ci % 2) * 2 * W:((ci % 2) + 1) * 2 * W],
                    tl[:, 0:2 * W],
                    mybir.ActivationFunctionType.Gelu,
                    bias=btail[:],
                    scale=1.0,
                )
                if ci % 2 == 1 and KPHASE >= 74:
                    r0 = 2 * (ci - 1)
                    nc.sync.dma_start(out_d[:, r0 * W:(r0 + 4) * W], ost[:])

            # ---- segments: products + accumulate + gelu ----
            seg_list = SEGS if KPHASE >= 3 else []
            # 49 taps per segment in kj-major order; interleaved extras:
            #   seg0 -> kern rows 24-47; seg1 -> tails of rows 0-23;
            #   seg2 -> tails of rows 24-39; trailing -> tails of rows 40-47
            for s, (r0, nr) in enumerate(seg_list):
                fd = nr * C
                atag = "accA" if s % 2 == 0 else "accB"
                acc = accp.tile([96, fd], F32, name=atag, tag=atag)
                for c0 in range(0, fd, 512):
                    nc.tensor.matmul(
                        acc[:, c0:c0 + 512], ones1[:],
                        birow[:, c0:c0 + 512], start=True, stop=False,
                    )
                acc_pending = []
                lag = [0]

                def flush_acc(limit):
                    while acc_pending and len(acc_pending) > limit:
                        pr_, last_ = acc_pending.pop(0)
                        for c0 in range(0, fd, 512):
                            nc.tensor.matmul(
                                acc[:, c0:c0 + 512], iden[:],
                                pr_[:, c0:c0 + 512],
                                start=False, stop=last_,
                            )

                for tapn in range(KK):
                    ki, kj = TAP_ORDER[tapn]
                    kk = ki * KS + kj
                    pool_tap = tapn in POOL_SLOTS and KPHASE >= 4
                    pp = prodP if pool_tap else prodD
                    pr = pp.tile([96, fd], BF16, name="prod", tag="prod")
                    pr4 = pr[:].rearrange("p (i u g) -> p i u g", i=nr, u=GC)
                    in0 = xs[kj][:].rearrange(
                        "p (i c) -> p i c", i=IROWS
                    )[:, r0 + ki:r0 + ki + nr, :].rearrange(
                        "p i (u g) -> p i u g", g=G
                    )
                    in1 = kern_pm[:].rearrange(
                        "p (i k) -> p i k", k=KK * G
                    )[:, r0:r0 + nr, kk * G:(kk + 1) * G]
                    in1b = in1.unsqueeze(2).to_broadcast([96, nr, GC, G])
                    if pool_tap:
                        nc.gpsimd.tensor_tensor(
                            out=pr4, in0=in0, in1=in1b,
                            op=mybir.AluOpType.mult,
                        )
                    else:
                        nc.vector.tensor_tensor(
                            out=pr4, in0=in0, in1=in1b,
                            op=mybir.AluOpType.mult,
                        )
                    # defer pool-tap accs ~2 slots so the in-order PE
                    # stream is not blocked by Pool's longer op latency
                    acc_pending.append((pr, tapn == KK - 1))
                    if pool_tap and int(os.environ.get("KLAG", "1")):
                        lag[0] = 2
                    flush_acc(lag[0])
                    if lag[0] > 0 and not pool_tap:
                        lag[0] -= 1
                    # interleaved kern production rows 24-47 (seg 0)
                    if (s == 0 and tapn % 4 == 1 and tapn // 4 < 12
                            and KPHASE >= 5):
                        kern_pair(12 + tapn // 4, kpsB, dve_copy=False)
                    # interleaved, phase-split tail chunks (segs 1-2)
                    if s == 1 and KPHASE >= 71:
                        if tapn % 4 == 1 and tapn // 4 < 12:
                            tail_a(tapn // 4, x1c_dve=False)
                        if tapn % 4 == 3 and tapn // 4 < 12:
                            tail_b(tapn // 4)
                    if s == 2 and KPHASE >= 71:
                        if tapn % 6 == 1 and tapn // 6 < 8:
                            tail_a(12 + tapn // 6, x1c_dve=False)
                        if tapn % 6 == 4 and tapn // 6 < 8:
                            tail_b(12 + tapn // 6)
                flush_acc(0)
                if KPHASE >= 6:
                    nc.scalar.activation(
                        x1g[:, r0 * C:(r0 + nr) * C],
                        acc[:],
                        mybir.ActivationFunctionType.Gelu,
                    )
            # trailing tail chunks (rows 40-47)
            if KPHASE >= 71:
                for ci in range(20, 24):
                    tail_a(ci, x1c_dve=True)
                    tail_b(ci)
            if KDEBUG:
                nc.sync.dma_start(tdbg_d[:, :], t_sb[:].bitcast(F32))
                nc.sync.dma_start(kdbg_d[:, :], kern_pm[:].bitcast(F32))
                nc.sync.dma_start(xdbg_d[:, :], x1g[:].bitcast(F32))
            tp2_cm.__exit__(None, None, None)
            tail_cm.__exit__(None, None, None)
            acc_cm.__exit__(None, None, None)
            kpsB_cm.__exit__(None, None, None)

    if not nc.is_finalized():
        nc.finalize()
    return nc


def _bf16_pack(arr):
    """bf16-cast a [P, N] array and pack into [P, N/2] f32 slots."""
    import ml_dtypes

    a = np.ascontiguousarray(np.asarray(arr, np.float32)).astype(ml_dtypes.bfloat16)
    return a.view(np.float32)


def _prep_blob_consts(w_reduce, g_r, b_r, m_r, v_r, w_span, b_span,
                      g_i, b_i, m_i, v_i, w_conv, g_c, b_c, m_c, v_c,
                      w_map, b_map, g_m, b_m, m_m, v_m, perm):
    f = np.float32

    def bn_fold(g, b, m, v):
        s = g / np.sqrt(v + EPS)
        return s.astype(f), (b - m * s).astype(f)

    sc_r, bi_r = bn_fold(g_r, b_r, m_r, v_r)
    sc_i, bi_i = bn_fold(g_i, b_i, m_i, v_i)
    sc_c, bi_c = bn_fold(g_c, b_c, m_c, v_c)
    sc_m, bi_m = bn_fold(g_m, b_m, m_m, v_m)

    cb = np.zeros((128, BLOBW), f)

    def put(name, arr, packed=False):
        a, b_ = _OFF[name]
        arr = np.asarray(arr, f)
        v = _bf16_pack(arr) if packed else arr
        cb[0:v.shape[0], a:a + v.shape[1]] = v

    # wspan_aug [17, 196]: rows 0-15 w_span, row 16 b_span; col = kk*4+g
    wsa = np.zeros((17, KK * G), f)
    ws3 = w_span.reshape(G, KK, CR)          # [g, kk, r]
    wsa[0:16] = ws3.transpose(2, 1, 0).reshape(CR, KK * G)
    wsa[16] = b_span.reshape(G, KK).T.reshape(KK * G)
    put("wspan", wsa, packed=True)

    # w_reduce^T with c' permutation, same 16 cols for both 64-partition halves
    wrt = np.zeros((128, CR), f)
    wrt[0:64] = w_reduce[:, perm].T
    wrt[64:128] = w_reduce[:, perm].T
    put("wredt", wrt, packed=True)
    put("screl", np.stack([sc_r, bi_r], axis=1))

    wc = (w_conv[:, perm] * sc_c[:, None]).T
    wm = (w_map[:, perm] * sc_m[:, None]).T
    put("wce", np.concatenate([wc, wm], axis=0), packed=True)
    put("wco", np.concatenate([wm, wc], axis=0), packed=True)
    put("btail", (bi_c + sc_m * b_map + bi_m)[:, None])
    put("birow", np.tile(bi_i[perm], HALF)[None, :], packed=True)
    put("ones", np.ones((1, ROWS * W), f), packed=True)
    put("iden", np.eye(96, dtype=f), packed=True)
    return cb, sc_i


def kernel(**inputs):
    x = np.asarray(inputs["x"], dtype=np.float32)
    assert x.shape == (B, C, H, W)

    # channel permutation c' = u*4 + g  (group innermost)
    perm = np.array([(cp // G) + GC * (cp % G) for cp in range(C)], np.int64)

    if "cb" not in _CACHE:
        cb, sc_i = _prep_blob_consts(
            **{k: np.asarray(v) for k, v in inputs.items() if k != "x"}, perm=perm
        )
        _CACHE["cb"] = cb
        _CACHE["sc_i"] = sc_i
    cb, sc_i = _CACHE["cb"], _CACHE["sc_i"]

    if "nc" not in _CACHE:
        _CACHE["nc"] = _build_bass()
    nc = _CACHE["nc"]

    xp = x[:, perm, :, :]                       # [B, c', H, W]
    xs_scaled = xp * sc_i[perm][None, :, None, None]

    in_maps = []
    for core in range(NCORES):
        b, half = core // 2, core % 2
        r0 = half * ROWS
        blob = cb.copy()
        # pixel-major x*s_i: [102 j, (54 i, 64 c')]
        xpm = np.zeros((JP, IROWS, C), np.float32)
        glo, ghi = max(r0 - PAD, 0), min(r0 + ROWS + PAD, H)
        xpm[PAD:PAD + W, glo - (r0 - PAD):ghi - (r0 - PAD), :] = (
            xs_scaled[b, :, glo:ghi, :].transpose(2, 1, 0)
        )
        a0, a1 = _OFF["xpm"]
        blob[0:JP, a0:a1] = _bf16_pack(xpm.reshape(JP, IROWS * C))
        # channel-major x: [128 = (2 half-blocks, 64 c'), 24*96]
        xcm = np.empty((128, HALF * W), np.float32)
        xcm[0:64] = xp[b, :, r0:r0 + HALF, :].reshape(C, HALF * W)
        xcm[64:128] = xp[b, :, r0 + HALF:r0 + ROWS, :].reshape(C, HALF * W)
        a0, a1 = _OFF["xcm"]
        blob[0:128, a0:a1] = _bf16_pack(xcm)
        # row-parity x for the tail map branch (E at partitions 64-127)
        xr = xp[b, :, r0:r0 + ROWS, :]              # [c', 48, 96]
        a0, a1 = _OFF["xcmE"]
        blob[64:128, a0:a1] = _bf16_pack(xr[:, 0::2, :].reshape(C, HALF * W))
        a0, a1 = _OFF["xcmO"]
        blob[0:64, a0:a1] = _bf16_pack(xr[:, 1::2, :].reshape(C, HALF * W))
        in_maps.append({"blob": blob})

    res = run_bass_kernel_spmd(nc, in_maps, core_ids=list(range(NCORES)))

    out = np.empty((B, CO, H, W), np.float32)
    for core in range(NCORES):
        b, half = core // 2, core % 2
        o = np.asarray(res.results[core]["out"]).astype(np.float32)
        out[b, :, half * ROWS:(half + 1) * ROWS, :] = o.reshape(CO, ROWS, W)
    return out
